# revision 1
# baseline (speedup 1.0000x reference)
"""Trainium2 Bass kernel for fused attention prefill (nn_Attn_50740743635107).

Reference computation (fp32):
  qkv = x @ W_qkv.T ; split q,k,v ; interleaved RoPE on q,k ;
  scores = q k^T / sqrt(dh) with causal+valid_k mask ; softmax ;
  ctx = attn @ v ; out = ctx @ W_out.T

Shapes: B=4, S=1024, D=2048, H=16, DH=128.

Sharding: 8 cores = 4 batches x 2 head-groups (8 heads each).
Each core computes a partial out^T [D, S] for its (batch, head-group);
the host sums the two head-group partials per batch and transposes.

Device-side design (zero on-device transposes):
- Host passes x^T and W^T layouts so every matmul contraction dim lands on
  SBUF partitions.
- q/k rows of W_qkv are permuted on host so RoPE even/odd interleave becomes
  contiguous halves (rows 0-63 = even dh, 64-127 = odd dh). The permutation
  cancels in q.k^T. The 1/sqrt(dh) scale is folded into W_q.
- scores are computed transposed (scoresT[sk, sq] = kT.T @ qT), masked with
  host-baked additive tiles, exponentiated without max-subtraction (scores
  are ~N(0, 0.5), so exp overflow is impossible), and both the ctx
  accumulation and the softmax denominators are computed with PE matmuls
  (denominator via an all-ones stationary operand, giving the denominator
  broadcast across partitions for a plain elementwise normalize).
- Causal structure skips 4 of 16 score tiles per head.
- Matmuls run in float32r (full PE rate at N>=256) except attn/v which are
  bf16.
"""

import numpy as np
import ml_dtypes

import concourse.bass as bass
from concourse import bacc
import concourse.mybir as mybir
import concourse.tile as tile
from concourse.bass_utils import run_bass_kernel_spmd

B, S, D, H = 4, 1024, 2048, 16
DH = 128           # head dim
HPC = 8            # heads per core
DC = HPC * DH      # 1024: d-range per core
P = 128
THETA = 10000.0
NEG = -60.0        # additive mask for disallowed positions
F32 = mybir.dt.float32
BF16 = mybir.dt.bfloat16
F32R = mybir.dt.float32r
MULT = mybir.AluOpType.mult
ADD = mybir.AluOpType.add
SUB = mybir.AluOpType.subtract
EXP = mybir.ActivationFunctionType.Exp

# score tiles per head: (sk_tile, sq_half) pairs that are (partially) allowed.
# sq_half h covers sq in [512h, 512h+512); sk tile t covers sk in [128t, 128t+128).
# partial (need mask): h=0: t=0..3 ; h=1: t=4..7.  full (no mask): h=1: t=0..3.
PARTIAL = {(t, 0) for t in range(4)} | {(t, 1) for t in range(4, 8)}
ALLOWED = {0: [0, 1, 2, 3], 1: [0, 1, 2, 3, 4, 5, 6, 7]}  # sq_half -> sk tiles


def build_nc(phases="ABO", reps=1):
    """phases: subset of 'A' (qkv+rope), 'B' (attention), 'O' (out proj).
    reps>1 repeats the whole body (for HW timing amortization)."""
    nc = bacc.Bacc()
    xT_d = nc.dram_tensor("xT", [D, S], F32R, kind="ExternalInput")
    w1T_d = nc.dram_tensor("w1T", [D, 3 * DC], F32R, kind="ExternalInput")
    woT_d = nc.dram_tensor("woT", [DC, D], F32R, kind="ExternalInput")
    cs_d = nc.dram_tensor("cs", [P, 2, S], F32R, kind="ExternalInput")
    tri_d = nc.dram_tensor("tri", [P, P], F32, kind="ExternalInput")
    bias_d = nc.dram_tensor("bias", [P, 8], F32, kind="ExternalInput")
    ones_d = nc.dram_tensor("ones", [P, P], BF16, kind="ExternalInput")
    outT_d = nc.dram_tensor("outT", [D, S], F32, kind="ExternalOutput")

    with tile.TileContext(nc) as tc:
      for rep in range(reps):
        with (
            tc.tile_pool(name="qkt", bufs=1) as qktp,      # [128,16,1024] f32 64K/p
            tc.tile_pool(name="vsb", bufs=1) as vsbp,      # [128,8,1024] bf16 16K/p
            tc.tile_pool(name="cstb", bufs=1) as cstbp,    # tri/bias/ones consts
            tc.tile_pool(name="ps", bufs=3, space=bass.MemorySpace.PSUM) as psp,
        ):
            qkT = qktp.tile([P, 16, S], F32R, tag="qkt")
            vsb = vsbp.tile([P, 8, DC], BF16, tag="vsb")
            tri_t = cstbp.tile([P, P], F32, tag="tri")
            nc.sync.dma_start(tri_t[:], tri_d[:])
            bias_t = cstbp.tile([P, 8], F32, tag="bias")
            nc.sync.dma_start(bias_t[:], bias_d[:])
            ones_t = cstbp.tile([P, P], BF16, tag="ones")
            nc.sync.dma_start(ones_t[:], ones_d[:])

            # ================= phase A: QKV projection + RoPE =================
            with (
                tc.tile_pool(name="xt", bufs=1) as xtp,      # [128,16,1024] f32 64K/p
                tc.tile_pool(name="wqk", bufs=2) as wqkp,    # [128,16,128] f32 8K/p
                tc.tile_pool(name="wv", bufs=2) as wvp,      # [128,16,256] f32 16K/p
                tc.tile_pool(name="cst", bufs=1) as cstp,    # cos/sin 8K/p
                tc.tile_pool(name="rope", bufs=1) as ropep,  # [128,1024] f32 4K/p
                tc.tile_pool(name="psv", bufs=2, space=bass.MemorySpace.PSUM) as psvp,
            ):
                xt = xtp.tile([P, 16, S], F32R, tag="xt")
                wqks = []
                with tc.high_priority():
                    # first weight tile in per-kt chunks on the ACT queue so
                    # the first matmuls are not gated on a full 1MB transfer
                    wqk0 = wqkp.tile([P, 16, P], F32R, tag="wqk", name="wqk_pre0")
                    for kt in range(16):
                        nc.scalar.dma_start(
                            wqk0[:, kt, :],
                            w1T_d[P * kt : P * (kt + 1), 0:P],
                        )
                    wqks.append(wqk0)
                    wqk1 = wqkp.tile([P, 16, P], F32R, tag="wqk", name="wqk_pre1")
                    nc.scalar.dma_start(
                        wqk1[:], w1T_d[:, P : 2 * P].rearrange("(t p) m -> p t m", p=P)
                    )
                    wqks.append(wqk1)
                    for kt in range(16):
                        eng = nc.sync if kt % 2 == 0 else nc.gpsimd
                        eng.dma_start(xt[:, kt, :], xT_d[P * kt : P * (kt + 1), :])
                cs_t = cstp.tile([P, 2, S], F32R, tag="cs")
                nc.scalar.dma_start(cs_t[:], cs_d[:])

                # ---- q/k projection (m-tile mt: 0..7 = q heads, 8..15 = k heads)
                # qkT[m, s] = sum_d w1T[d, m] * xT[d, s]
                for mt in range(16 if "A" in phases else 0):
                    if mt < 2:
                        wqk = wqks[mt]
                    else:
                        wqk = wqkp.tile([P, 16, P], F32R, tag="wqk")
                        nc.scalar.dma_start(
                            wqk[:],
                            w1T_d[:, P * mt : P * (mt + 1)].rearrange(
                                "(t p) m -> p t m", p=P
                            ),
                        )
                    ps0 = psp.tile([P, 512], F32, tag="ps", name=f"qk{mt}_0")
                    ps1 = psp.tile([P, 512], F32, tag="ps", name=f"qk{mt}_1")
                    for kt in range(16):
                        nc.tensor.matmul(
                            ps0[:], wqk[:, kt, :], xt[:, kt, 0:512],
                            start=(kt == 0), stop=(kt == 15),
                        )
                        nc.tensor.matmul(
                            ps1[:], wqk[:, kt, :], xt[:, kt, 512:1024],
                            start=(kt == 0), stop=(kt == 15),
                        )
                    nc.vector.tensor_copy(qkT[:, mt, 0:512], ps0[:])
                    nc.vector.tensor_copy(qkT[:, mt, 512:1024], ps1[:])
                    # ---- RoPE in place on qkT[:, mt, :] once both halves done.
                    # rows 0..63 = even dh (xe), 64..127 = odd dh (xo):
                    #   new_e = xe*cos - xo*sin ; new_o = xe*sin + xo*cos
                    tmp = ropep.tile([P, S], F32R, tag="rope")
                    col = qkT[:, mt, :]
                    nc.vector.tensor_tensor(
                        tmp[0:64, :], col[64:128, :], cs_t[64:128, 1, :], op=MULT
                    )
                    nc.vector.tensor_tensor(
                        tmp[64:128, :], col[0:64, :], cs_t[0:64, 1, :], op=MULT
                    )
                    nc.vector.tensor_tensor(col[:], col[:], cs_t[:, 0, :], op=MULT)
                    nc.vector.tensor_tensor(
                        col[0:64, :], col[0:64, :], tmp[0:64, :], op=SUB
                    )
                    nc.vector.tensor_tensor(
                        col[64:128, :], col[64:128, :], tmp[64:128, :], op=ADD
                    )

                # ---- v projection: v[s, dh] = sum_d xT[d, s] * w1T[d, 2048+dh]
                for nh in range(4 if "A" in phases else 0):
                    wv = wvp.tile([P, 16, 256], F32R, tag="wv")
                    nc.gpsimd.dma_start(
                        wv[:],
                        w1T_d[:, 2 * DC + 256 * nh : 2 * DC + 256 * (nh + 1)].rearrange(
                            "(t p) m -> p t m", p=P
                        ),
                    )
                    for st in range(8):
                        psv = psvp.tile([P, 256], F32, tag="psv")
                        for kt in range(16):
                            nc.tensor.matmul(
                                psv[:],
                                xt[:, kt, P * st : P * (st + 1)],
                                wv[:, kt, :],
                                start=(kt == 0),
                                stop=(kt == 15),
                            )
                        nc.scalar.copy(vsb[:, st, 256 * nh : 256 * (nh + 1)], psv[:])

            # ============ phase B: attention + output projection ============
            with (
                tc.tile_pool(name="ctx", bufs=1) as ctxp,    # [128,8,1024] f32 32K/p
                tc.tile_pool(name="ex", bufs=4) as exps,     # [128,512] bf16 1K/p
                tc.tile_pool(name="msc", bufs=2) as mscp,    # [128,128] f32 0.5K/p
                tc.tile_pool(name="rc", bufs=1) as rcp,      # [128,512] f32 2K/p
                tc.tile_pool(name="wo", bufs=1) as wop,      # [128,8,2048] f32 64K/p
                tc.tile_pool(name="ot", bufs=2) as otp,      # [128,512] f32 2K/p
                tc.tile_pool(name="psc", bufs=2, space=bass.MemorySpace.PSUM) as pscp,
                tc.tile_pool(name="psd", bufs=2, space=bass.MemorySpace.PSUM) as psdp,
            ):
                ctxT = ctxp.tile([P, 8, S], F32R, tag="ctx")
                # W_out^T resident for the output projection
                wo_t = wop.tile([P, 8, D], F32R, tag="wo")
                nc.sync.dma_start(wo_t[:], woT_d.rearrange("(t p) e -> p t e", p=P))

                # ---- attention, software-pipelined: scores issue LOOKAHEAD
                # items ahead of their exp/ctx/den so the PE never waits on
                # the DVE/ACT exp chain.
                work = []  # (h, sh, t, i, ntiles)
                for h in range(8 if "B" in phases else 0):
                    for sh in range(2):
                        tiles = ALLOWED[sh]
                        for i, t in enumerate(tiles):
                            work.append((h, sh, t, i, len(tiles)))

                LOOKAHEAD = 2
                scs = {}
                groups = {}  # (h, sh) -> (ctx_ps, den_ps)

                def issue_score(j):
                    h, sh, t, i, _n = work[j]
                    partial = (t, sh) in PARTIAL
                    c0 = P * t - 512 * sh if partial else 0
                    sc = psp.tile([P, 512], F32, tag="ps")
                    nc.tensor.matmul(
                        sc[:, c0:512],
                        qkT[:, 8 + h, P * t : P * (t + 1)],
                        qkT[:, h, 512 * sh + c0 : 512 * (sh + 1)],
                        start=True,
                        stop=True,
                    )
                    scs[j] = sc

                for j in range(min(LOOKAHEAD, len(work))):
                    issue_score(j)
                for j, (h, sh, t, i, ntiles) in enumerate(work):
                    if j + LOOKAHEAD < len(work):
                        issue_score(j + LOOKAHEAD)
                    sc = scs.pop(j)
                    partial = (t, sh) in PARTIAL
                    c0 = P * t - 512 * sh if partial else 0
                    ex = exps.tile([P, 512], BF16, tag="ex")
                    if partial:
                        bias = bias_t[:, t : t + 1] if sh == 1 else 0.0
                        msc = mscp.tile([P, P], F32, tag="msc")
                        nc.vector.tensor_tensor(
                            msc[:], sc[:, c0 : c0 + P], tri_t[:], op=ADD
                        )
                        nc.scalar.activation(ex[:, c0 : c0 + P], msc[:], EXP, bias=bias)
                        if c0 + P < 512:
                            nc.scalar.activation(
                                ex[:, c0 + P : 512], sc[:, c0 + P : 512], EXP, bias=bias
                            )
                    else:
                        nc.scalar.activation(ex[:], sc[:], EXP)
                    if i == 0:
                        ctx_ps = pscp.tile([P, 512], F32, tag="psc", name=f"ctxps_{h}_{sh}")
                        den_ps = psdp.tile([P, 512], F32, tag="psd", name=f"denps_{h}_{sh}")
                        groups[(h, sh)] = (ctx_ps, den_ps)
                    ctx_ps, den_ps = groups[(h, sh)]
                    first, last = (i == 0), (i == ntiles - 1)
                    nc.tensor.matmul(
                        ctx_ps[:, c0:512],
                        vsb[:, t, DH * h : DH * (h + 1)],
                        ex[:, c0:512],
                        start=first,
                        stop=last,
                    )
                    nc.tensor.matmul(
                        den_ps[:, c0:512], ones_t[:], ex[:, c0:512], start=first, stop=last
                    )
                    if last:
                        rc = rcp.tile([P, 512], F32, tag="rc")
                        nc.vector.reciprocal(rc[:], den_ps[:])
                        nc.vector.tensor_tensor(
                            ctxT[:, h, 512 * sh : 512 * (sh + 1)],
                            ctx_ps[:],
                            rc[:],
                            op=MULT,
                        )

                # ---- output projection: outT[e, sq] = sum_d woT[d, e] * ctxT[d, sq]
                for me in range(16 if "O" in phases else 0):
                    for sh in range(2):
                        ps = psp.tile([P, 512], F32, tag="ps")
                        for kd in range(8):
                            nc.tensor.matmul(
                                ps[:],
                                wo_t[:, kd, P * me : P * (me + 1)],
                                ctxT[:, kd, 512 * sh : 512 * (sh + 1)],
                                start=(kd == 0),
                                stop=(kd == 7),
                            )
                        ot = otp.tile([P, 512], F32, tag="ot")
                        nc.scalar.copy(ot[:], ps[:])
                        nc.sync.dma_start(
                            outT_d[P * me : P * (me + 1), 512 * sh : 512 * (sh + 1)],
                            ot[:],
                        )
    nc.finalize()
    return nc


_NC_CACHE = None


def get_nc():
    global _NC_CACHE
    if _NC_CACHE is None:
        _NC_CACHE = build_nc()
    return _NC_CACHE


def make_in_maps(in_features, attention_mask, W_qkv, W_out):
    x = np.asarray(in_features, np.float32)
    am = np.asarray(attention_mask)
    Wqkv = np.asarray(W_qkv, np.float32)
    Wout = np.asarray(W_out, np.float32)
    seq_lens = am.astype(np.int64).sum(-1)

    perm = np.concatenate([np.arange(0, DH, 2), np.arange(1, DH, 2)])
    Wqh = Wqkv[0:D].reshape(H, DH, D)
    Wkh = Wqkv[D : 2 * D].reshape(H, DH, D)
    Wvh = Wqkv[2 * D : 3 * D].reshape(H, DH, D)
    scale = DH**-0.5

    half = DH // 2
    freq = THETA ** (-2.0 * np.arange(half, dtype=np.float64) / DH)
    ang = np.arange(S, dtype=np.float64)[:, None] * freq  # [S, 64]
    cosv = np.cos(ang).T.astype(np.float32)  # [64, S]
    sinv = np.sin(ang).T.astype(np.float32)
    cs = np.empty([P, 2, S], np.float32)
    cs[0:64, 0] = cosv
    cs[64:128, 0] = cosv
    cs[0:64, 1] = sinv
    cs[64:128, 1] = sinv

    ones = np.ones([P, P], ml_dtypes.bfloat16)

    in_maps = []
    for c in range(8):
        b, g = c // 2, c % 2
        hs = slice(g * HPC, (g + 1) * HPC)
        wq = (Wqh[hs][:, perm, :] * scale).reshape(DC, D)
        wk = Wkh[hs][:, perm, :].reshape(DC, D)
        wv = Wvh[hs].reshape(DC, D)
        w1T = np.ascontiguousarray(np.concatenate([wq, wk, wv], 0).T)  # [D, 3DC]
        xT = np.ascontiguousarray(x[b].T)  # [D, S]
        woT = np.ascontiguousarray(Wout[:, g * DC : (g + 1) * DC].T)  # [DC, D]

        sl = int(seq_lens[b])
        pp = np.arange(P)[:, None]
        cc = np.arange(P)[None, :]
        tri = np.where(pp <= cc, 0.0, NEG).astype(np.float32)
        bias = np.zeros([P, 8], np.float32)
        for t in range(4, 8):
            bias[:, t] = np.where(t * P + np.arange(P) >= sl, NEG, 0.0)
        in_maps.append(
            dict(
                xT=xT,
                w1T=w1T,
                woT=woT,
                cs=cs,
                tri=tri,
                bias=bias,
                ones=ones,
            )
        )
    return in_maps


def kernel(in_features, past_k, past_v, attention_mask, W_qkv, W_out):
    nc = get_nc()
    in_maps = make_in_maps(in_features, attention_mask, W_qkv, W_out)
    res = run_bass_kernel_spmd(nc, in_maps, core_ids=list(range(8)))
    out = np.empty((B, S, D), np.float32)
    for b in range(B):
        out[b] = (res.results[2 * b]["outT"] + res.results[2 * b + 1]["outT"]).T
    return out



# revision 7
# speedup vs baseline: 1.4410x; 1.4410x over previous
"""Trainium2 Bass kernel for fused attention prefill (nn_Attn_50740743635107).

Reference computation (fp32):
  qkv = x @ W_qkv.T ; split q,k,v ; interleaved RoPE on q,k ;
  scores = q k^T / sqrt(dh) with causal+valid_k mask ; softmax ;
  ctx = attn @ v ; out = ctx @ W_out.T

Shapes: B=4, S=1024, D=2048, H=16, DH=128.

Sharding: 8 cores = 4 batches x 2 head-groups (8 heads each).
Each core computes a partial out^T for its (batch, head-group);
the host sums the two head-group partials per batch and transposes.

v2 design (vs fp32r v1):
- All operands bf16 except the q/k projection which runs fp8e4m3 with
  DoubleRow perf mode (2 k-tiles per matmul). Host bakes power-of-2
  scales (x*8, Wqk*64); the combined descale folds into the softmax
  exp's free scale operand.
- Host pre-arranges every DRAM tensor into its exact SBUF layout so all
  DMAs are contiguous.
- RoPE in bf16 on DVE/GpSimd (4 tensor_tensor ops per 128-row tile, 2x
  DVE mode); q/k rows pre-permuted so even/odd pairs become contiguous
  halves.
- Causal mask applied multiplicatively AFTER exp (0/1 bf16 tri tile) so
  exp reads score PSUM directly; ragged seq_len masking stays as a
  per-partition additive bias inside the exp activation.
- Softmax reciprocal via ACT ln + exp(-x) (both functions live in the
  natural_log_exp table set) instead of the slow DVE iterative divide.
- Denominators via an all-ones stationary matmul accumulated alongside
  ctx, giving a partition-broadcast denominator for the normalize.
"""

import numpy as np
import ml_dtypes

import concourse.bass as bass
from concourse import bacc
from concourse import hw_specs
import concourse.mybir as mybir
import concourse.tile as tile
from concourse.bass_utils import run_bass_kernel_spmd

B, S, D, H = 4, 1024, 2048, 16
DH = 128           # head dim
HPC = 8            # heads per core
DC = HPC * DH      # 1024: d-range per core
P = 128
THETA = 10000.0
NEG = -60.0
F32 = mybir.dt.float32
BF16 = mybir.dt.bfloat16
FP8 = mybir.dt.float8e4
MULT = mybir.AluOpType.mult
ADD = mybir.AluOpType.add
SUB = mybir.AluOpType.subtract
EXP = mybir.ActivationFunctionType.Exp
LN = mybir.ActivationFunctionType.Ln
COPY = mybir.ActivationFunctionType.Copy
DR = mybir.MatmulPerfMode.DoubleRow

FP8_QK = True      # fp8 DoubleRow q/k projection (bf16 fallback if False)
SX, SW = 8.0, 64.0
_F = (SX * SW) if FP8_QK else 1.0
SCALE = float(1.0 / (_F * _F * np.sqrt(DH)))   # softmax exp input scale

# score tiles per head: ALLOWED[sq_half] = sk tiles; diagonal tiles are
# masked via the 0/1 tri tile, (t>=4, sh=1) also get the seq-len bias.
ALLOWED = {0: [0, 1, 2, 3], 1: [0, 1, 2, 3, 4, 5, 6, 7]}
PARTIAL = {(t, 0) for t in range(4)} | {(t, 1) for t in range(4, 8)}


def build_nc(reps=1):
    nc = bacc.Bacc()
    qk_dt = FP8 if FP8_QK else BF16
    x8_d = nc.dram_tensor("x8", [P, 16, S], qk_dt, kind="ExternalInput")
    xb_d = nc.dram_tensor("xb", [P, 16, S], BF16, kind="ExternalInput")
    w8_d = nc.dram_tensor("w8", [P, 16, 16, P], qk_dt, kind="ExternalInput")
    wv_d = nc.dram_tensor("wv", [P, 2, 16, 512], BF16, kind="ExternalInput")
    wo_d = nc.dram_tensor("wo", [P, 8, D], BF16, kind="ExternalInput")
    cs_d = nc.dram_tensor("cs", [P, 2, S], BF16, kind="ExternalInput")
    tri_d = nc.dram_tensor("tri", [P, P], BF16, kind="ExternalInput")
    bias_d = nc.dram_tensor("bias", [P, 4], F32, kind="ExternalInput")
    ones_d = nc.dram_tensor("ones", [P, P], BF16, kind="ExternalInput")
    outT_d = nc.dram_tensor("outT", [P, 16, S], BF16, kind="ExternalOutput")

    with tile.TileContext(nc) as tc:
      for rep in range(reps):
        with (
            tc.tile_pool(name="qkt", bufs=1) as qktp,      # [128,16,1024] bf16 32K/p
            tc.tile_pool(name="vsb", bufs=1) as vsbp,      # [128,8,1024] bf16 16K/p
            tc.tile_pool(name="cstb", bufs=1) as cstbp,    # tri/bias/ones consts
            tc.tile_pool(name="ps", bufs=3, space=bass.MemorySpace.PSUM) as psp,
        ):
            # Pin the ACT table to natural_log_exp_and_others (contains Exp,
            # Ln, Copy) so the auto-insertion pass doesn't ping-pong between
            # the exp-only and ln-only sets (~2.7us per reload).
            _tabs = list(hw_specs.get_activation_tables(nc.m.arch).keys())
            _ld = mybir.InstLoadActFuncSet(
                name=nc.get_next_instruction_name(), ins=[], outs=[],
                act_func_set_id=_tabs.index("natural_log_exp_and_others"),
            )
            _ld.engine = mybir.EngineType.Activation
            nc.scalar.add_instruction(_ld)

            qkT = qktp.tile([P, 16, S], BF16, tag="qkt")
            vsb = vsbp.tile([P, 8, DC], BF16, tag="vsb")
            tri_t = cstbp.tile([P, P], BF16, tag="tri")
            nc.scalar.dma_start(tri_t[:], tri_d[:])
            bias_t = cstbp.tile([P, 4], F32, tag="bias")
            nc.scalar.dma_start(bias_t[:], bias_d[:])
            ones_t = cstbp.tile([P, P], BF16, tag="ones")
            nc.scalar.dma_start(ones_t[:], ones_d[:])

            # ================= phase A: QKV projection + RoPE =================
            with (
                tc.tile_pool(name="x8", bufs=1) as x8p,      # fp8 16K/p (bf16 32K)
                tc.tile_pool(name="xb", bufs=1) as xbp,      # bf16 32K/p
                tc.tile_pool(name="wqk", bufs=2) as wqkp,    # [128,16,128] 2K/p fp8
                tc.tile_pool(name="wv", bufs=2) as wvp,      # [128,16,512] 16K/p bf16
                tc.tile_pool(name="cst", bufs=1) as cstp,    # cos/sin 4K/p
                tc.tile_pool(name="rope", bufs=2) as ropep,  # [128,1024] bf16 2K/p
                tc.tile_pool(name="psv", bufs=2, space=bass.MemorySpace.PSUM) as psvp,
            ):
                x8t = x8p.tile([P, 16, S], qk_dt, tag="x8")
                xbt = xbp.tile([P, 16, S], BF16, tag="xb")
                wqks = []
                with tc.high_priority():
                    # first weight tile + x8 first: unblock the first matmuls
                    wqk0 = wqkp.tile([P, 16, P], qk_dt, tag="wqk", name="wqk_pre0")
                    nc.scalar.dma_start(wqk0[:], w8_d[:, 0])
                    wqks.append(wqk0)
                    wqk1 = wqkp.tile([P, 16, P], qk_dt, tag="wqk", name="wqk_pre1")
                    nc.scalar.dma_start(wqk1[:], w8_d[:, 1])
                    wqks.append(wqk1)
                    for half in range(2):
                        nc.sync.dma_start(
                            x8t[:, 8 * half : 8 * (half + 1), :],
                            x8_d[:, 8 * half : 8 * (half + 1), :],
                        )
                cs_t = cstp.tile([P, 2, S], BF16, tag="cs")
                nc.scalar.dma_start(cs_t[:], cs_d[:])
                for half in range(2):
                    nc.gpsimd.dma_start(
                        xbt[:, 8 * half : 8 * (half + 1), :],
                        xb_d[:, 8 * half : 8 * (half + 1), :],
                    )
                wvts = []
                for nhp in range(2):
                    wvt = wvp.tile([P, 16, 512], BF16, tag="wv", name=f"wv{nhp}")
                    nc.gpsimd.dma_start(wvt[:], wv_d[:, nhp])
                    wvts.append(wvt)

                # ---- q/k projection (m-tile mt: 0..7 = q heads, 8..15 = k heads)
                for mt in range(16):
                    if mt < 2:
                        wqk = wqks[mt]
                    else:
                        wqk = wqkp.tile([P, 16, P], qk_dt, tag="wqk")
                        nc.scalar.dma_start(wqk[:], w8_d[:, mt])
                    ps0 = psp.tile([P, 512], F32, tag="ps", name=f"qk{mt}_0")
                    ps1 = psp.tile([P, 512], F32, tag="ps", name=f"qk{mt}_1")
                    if FP8_QK:
                        for kk in range(8):
                            lw = wqk[:, 2 * kk : 2 * kk + 2, :]
                            nc.tensor.matmul(
                                ps0[:], lw, x8t[:, 2 * kk : 2 * kk + 2, 0:512],
                                start=(kk == 0), stop=(kk == 7), perf_mode=DR,
                            )
                            nc.tensor.matmul(
                                ps1[:], lw, x8t[:, 2 * kk : 2 * kk + 2, 512:1024],
                                start=(kk == 0), stop=(kk == 7), perf_mode=DR,
                            )
                    else:
                        for kt in range(16):
                            nc.tensor.matmul(
                                ps0[:], wqk[:, kt, :], x8t[:, kt, 0:512],
                                start=(kt == 0), stop=(kt == 15),
                            )
                            nc.tensor.matmul(
                                ps1[:], wqk[:, kt, :], x8t[:, kt, 512:1024],
                                start=(kt == 0), stop=(kt == 15),
                            )
                    col = qkT[:, mt, :]
                    nc.scalar.activation(col[:, 0:512], ps0[:], COPY)
                    nc.scalar.activation(col[:, 512:1024], ps1[:], COPY)
                    # ---- RoPE in place on qkT[:, mt, :].
                    # rows 0..63 = even dh (xe), 64..127 = odd dh (xo):
                    #   new_e = xe*cos - xo*sin ; new_o = xo*cos + xe*sin
                    # (both TT inputs must share a base partition; only the
                    # output AP may be partition-shifted)
                    eng = nc.gpsimd if mt % 4 == 3 else nc.vector
                    tmp = ropep.tile([P, S], BF16, tag="rope")
                    eng.tensor_tensor(
                        tmp[0:64, :], col[64:128, :], cs_t[64:128, 1, :], op=MULT
                    )
                    eng.tensor_tensor(
                        tmp[64:128, :], col[0:64, :], cs_t[0:64, 1, :], op=MULT
                    )
                    eng.tensor_tensor(col[:], col[:], cs_t[:, 0, :], op=MULT)
                    eng.tensor_tensor(
                        col[0:64, :], col[0:64, :], tmp[0:64, :], op=SUB
                    )
                    eng.tensor_tensor(
                        col[64:128, :], col[64:128, :], tmp[64:128, :], op=ADD
                    )

                # ---- v projection: v[s, j] = sum_d xb[d, s] * Wv[j, d]
                for nhp in range(2):
                    wvt = wvts[nhp]
                    for st in range(8):
                        psv = psvp.tile([P, 512], F32, tag="psv")
                        for kt in range(16):
                            nc.tensor.matmul(
                                psv[:],
                                xbt[:, kt, P * st : P * (st + 1)],
                                wvt[:, kt, :],
                                start=(kt == 0),
                                stop=(kt == 15),
                            )
                        nc.scalar.copy(vsb[:, st, 512 * nhp : 512 * (nhp + 1)], psv[:])

            # ============ phase B: attention + output projection ============
            with (
                tc.tile_pool(name="ctx", bufs=1) as ctxp,    # [128,8,1024] bf16 16K/p
                tc.tile_pool(name="ex", bufs=5) as exps,     # [128,512] bf16 1K/p
                tc.tile_pool(name="rc", bufs=4) as rcp,      # [128,512] f32 2K/p
                tc.tile_pool(name="wo", bufs=1) as wop,      # [128,8,2048] bf16 32K/p
                tc.tile_pool(name="ot", bufs=3) as otp,      # [128,512] bf16 1K/p
                tc.tile_pool(name="psc", bufs=2, space=bass.MemorySpace.PSUM) as pscp,
                tc.tile_pool(name="psd", bufs=2, space=bass.MemorySpace.PSUM) as psdp,
            ):
                ctxT = ctxp.tile([P, 8, S], BF16, tag="ctx")
                wo_t = wop.tile([P, 8, D], BF16, tag="wo")
                nc.sync.dma_start(wo_t[:], wo_d[:])

                # ---- attention, software-pipelined: scores issue LOOKAHEAD
                # items ahead of their exp/ctx/den so the PE never waits on
                # the ACT exp chain.
                work = []  # (h, sh, t, i, ntiles)
                for h in range(8):
                    for sh in range(2):
                        tiles = ALLOWED[sh]
                        for i, t in enumerate(tiles):
                            work.append((h, sh, t, i, len(tiles)))

                LOOKAHEAD = 2
                scs = {}
                groups = {}  # (h, sh) -> (ctx_ps, den_ps)

                def issue_score(j):
                    h, sh, t, i, _n = work[j]
                    partial = (t, sh) in PARTIAL
                    c0 = P * t - 512 * sh if partial else 0
                    sc = psp.tile([P, 512], F32, tag="ps")
                    nc.tensor.matmul(
                        sc[:, c0:512],
                        qkT[:, 8 + h, P * t : P * (t + 1)],
                        qkT[:, h, 512 * sh + c0 : 512 * (sh + 1)],
                        start=True,
                        stop=True,
                    )
                    scs[j] = sc

                for j in range(min(LOOKAHEAD, len(work))):
                    issue_score(j)
                for j, (h, sh, t, i, ntiles) in enumerate(work):
                    if j + LOOKAHEAD < len(work):
                        issue_score(j + LOOKAHEAD)
                    sc = scs.pop(j)
                    partial = (t, sh) in PARTIAL
                    c0 = P * t - 512 * sh if partial else 0
                    ex = exps.tile([P, 512], BF16, tag="ex")
                    bias = bias_t[:, t - 4 : t - 3] if (sh == 1 and t >= 4) else 0.0
                    nc.scalar.activation(
                        ex[:, c0:512], sc[:, c0:512], EXP, bias=bias, scale=SCALE
                    )
                    if partial:
                        nc.vector.tensor_tensor(
                            ex[:, c0 : c0 + P], ex[:, c0 : c0 + P], tri_t[:], op=MULT
                        )
                    if i == 0:
                        ctx_ps = pscp.tile([P, 512], F32, tag="psc", name=f"ctxps_{h}_{sh}")
                        den_ps = psdp.tile([P, 512], F32, tag="psd", name=f"denps_{h}_{sh}")
                        groups[(h, sh)] = (ctx_ps, den_ps)
                    ctx_ps, den_ps = groups[(h, sh)]
                    first, last = (i == 0), (i == ntiles - 1)
                    nc.tensor.matmul(
                        ctx_ps[:, c0:512],
                        vsb[:, t, DH * h : DH * (h + 1)],
                        ex[:, c0:512],
                        start=first,
                        stop=last,
                    )
                    nc.tensor.matmul(
                        den_ps[:, c0:512], ones_t[:], ex[:, c0:512], start=first, stop=last
                    )
                    if last:
                        # 1/den on ACT: rc = exp(-ln(den)); both fns share
                        # the natural_log_exp table set (no table switch).
                        lden = rcp.tile([P, 512], F32, tag="rc")
                        rc = rcp.tile([P, 512], F32, tag="rc")
                        nc.scalar.activation(lden[:], den_ps[:], LN)
                        nc.scalar.activation(rc[:], lden[:], EXP, scale=-1.0)
                        nc.vector.tensor_tensor(
                            ctxT[:, h, 512 * sh : 512 * (sh + 1)],
                            ctx_ps[:],
                            rc[:],
                            op=MULT,
                        )

                # ---- output projection: outT[e, sq] = sum_d wo[d, e] * ctxT[d, sq]
                for me in range(16):
                    for sh in range(2):
                        po = psp.tile([P, 512], F32, tag="ps")
                        for kd in range(8):
                            nc.tensor.matmul(
                                po[:],
                                wo_t[:, kd, P * me : P * (me + 1)],
                                ctxT[:, kd, 512 * sh : 512 * (sh + 1)],
                                start=(kd == 0),
                                stop=(kd == 7),
                            )
                        ot = otp.tile([P, 512], BF16, tag="ot")
                        if (me + sh) % 2 == 0:
                            nc.scalar.copy(ot[:], po[:])
                        else:
                            nc.vector.tensor_copy(ot[:], po[:])
                        eng = nc.sync if sh == 0 else nc.scalar
                        eng.dma_start(
                            outT_d[:, me, 512 * sh : 512 * (sh + 1)], ot[:]
                        )
    nc.finalize()
    return nc


_NC_CACHE = None


def get_nc():
    global _NC_CACHE
    if _NC_CACHE is None:
        _NC_CACHE = build_nc()
    return _NC_CACHE


def _bf16(a):
    return np.ascontiguousarray(a.astype(ml_dtypes.bfloat16))


def _fp8(a):
    return np.ascontiguousarray(
        np.clip(a, -240.0, 240.0).astype(ml_dtypes.float8_e4m3)
    )


def make_in_maps(in_features, attention_mask, W_qkv, W_out):
    x = np.asarray(in_features, np.float32)
    am = np.asarray(attention_mask)
    Wqkv = np.asarray(W_qkv, np.float32)
    Wout = np.asarray(W_out, np.float32)
    seq_lens = am.astype(np.int64).sum(-1)

    perm = np.concatenate([np.arange(0, DH, 2), np.arange(1, DH, 2)])
    Wqh = Wqkv[0:D].reshape(H, DH, D)
    Wkh = Wqkv[D : 2 * D].reshape(H, DH, D)
    Wvh = Wqkv[2 * D : 3 * D].reshape(H, DH, D)

    half = DH // 2
    freq = THETA ** (-2.0 * np.arange(half, dtype=np.float64) / DH)
    ang = np.arange(S, dtype=np.float64)[:, None] * freq  # [S, 64]
    cosv = np.cos(ang).T.astype(np.float32)  # [64, S]
    sinv = np.sin(ang).T.astype(np.float32)
    cs = np.empty([P, 2, S], np.float32)
    cs[0:64, 0] = cosv
    cs[64:128, 0] = cosv
    cs[0:64, 1] = sinv
    cs[64:128, 1] = sinv
    cs = _bf16(cs)

    ones = np.ones([P, P], ml_dtypes.bfloat16)
    pp = np.arange(P)[:, None]
    cc = np.arange(P)[None, :]
    tri = _bf16((pp <= cc).astype(np.float32))  # 1 on/above diag (sk<=sq)

    cvt_qk = _fp8 if FP8_QK else _bf16
    sw = SW if FP8_QK else 1.0
    sx = SX if FP8_QK else 1.0

    in_maps = []
    for c in range(8):
        b, g = c // 2, c % 2
        hs = slice(g * HPC, (g + 1) * HPC)
        # q/k weights: [p, mt, kt, m]; mt 0..7 q heads, 8..15 k heads
        wq = Wqh[hs][:, perm, :].reshape(DC, D) * sw
        wk = Wkh[hs][:, perm, :].reshape(DC, D) * sw
        wcat = np.concatenate([wq, wk], 0)               # [2048, 2048] (m, d)
        w8 = cvt_qk(wcat.reshape(16, P, 16, P).transpose(3, 0, 2, 1))
        # v weights: [p, nhp, kt, j]
        wv = Wvh[hs].reshape(DC, D)                      # [1024, 2048] (j, d)
        wv8 = _bf16(wv.reshape(2, 512, 16, P).transpose(3, 0, 2, 1))
        # out weights: [p, kd, e]
        wo8 = _bf16(
            Wout[:, g * DC : (g + 1) * DC].reshape(D, 8, P).transpose(2, 1, 0)
        )
        # x: [p, kt, s]
        xT = x[b].T                                      # [D, S]
        x8 = cvt_qk((xT * sx).reshape(16, P, S).transpose(1, 0, 2))
        xb = _bf16(xT.reshape(16, P, S).transpose(1, 0, 2))

        sl = int(seq_lens[b])
        bias = np.zeros([P, 4], np.float32)
        for t in range(4, 8):
            bias[:, t - 4] = np.where(t * P + np.arange(P) >= sl, NEG, 0.0)
        in_maps.append(
            dict(x8=x8, xb=xb, w8=w8, wv=wv8, wo=wo8, cs=cs,
                 tri=tri, bias=bias, ones=ones)
        )
    return in_maps


def kernel(in_features, past_k, past_v, attention_mask, W_qkv, W_out):
    nc = get_nc()
    in_maps = make_in_maps(in_features, attention_mask, W_qkv, W_out)
    res = run_bass_kernel_spmd(nc, in_maps, core_ids=list(range(8)))
    out = np.empty((B, S, D), np.float32)
    for b in range(B):
        acc = None
        for g in range(2):
            o = res.results[2 * b + g]["outT"].astype(np.float32)
            o = o.transpose(1, 0, 2).reshape(D, S)       # [e, s]
            acc = o if acc is None else acc + o
        out[b] = acc.T
    return out


# revision 15
# speedup vs baseline: 1.4455x; 1.0031x over previous
"""Trainium2 Bass kernel for fused attention prefill (nn_Attn_50740743635107).

Reference computation (fp32):
  qkv = x @ W_qkv.T ; split q,k,v ; interleaved RoPE on q,k ;
  scores = q k^T / sqrt(dh) with causal+valid_k mask ; softmax ;
  ctx = attn @ v ; out = ctx @ W_out.T

Shapes: B=4, S=1024, D=2048, H=16, DH=128.

Sharding: 8 cores = 4 batches x 2 head-groups (8 heads each).
Each core computes a partial out^T for its (batch, head-group);
the host sums the two head-group partials per batch and transposes.

v2 design (vs fp32r v1):
- All operands bf16 except the q/k projection which runs fp8e4m3 with
  DoubleRow perf mode (2 k-tiles per matmul). Host bakes power-of-2
  scales (x*8, Wqk*64); the combined descale folds into the softmax
  exp's free scale operand.
- Host pre-arranges every DRAM tensor into its exact SBUF layout so all
  DMAs are contiguous.
- RoPE in bf16 on DVE/GpSimd (4 tensor_tensor ops per 128-row tile, 2x
  DVE mode); q/k rows pre-permuted so even/odd pairs become contiguous
  halves.
- Causal mask applied multiplicatively AFTER exp (0/1 bf16 tri tile) so
  exp reads score PSUM directly; ragged seq_len masking stays as a
  per-partition additive bias inside the exp activation.
- Softmax reciprocal via ACT ln + exp(-x) (both functions live in the
  natural_log_exp table set) instead of the slow DVE iterative divide.
- Denominators via an all-ones stationary matmul accumulated alongside
  ctx, giving a partition-broadcast denominator for the normalize.
"""

import numpy as np
import ml_dtypes

import concourse.bass as bass
from concourse import bacc
from concourse import hw_specs
import concourse.mybir as mybir
import concourse.tile as tile
from concourse.bass_utils import run_bass_kernel_spmd

B, S, D, H = 4, 1024, 2048, 16
DH = 128           # head dim
HPC = 8            # heads per core
DC = HPC * DH      # 1024: d-range per core
P = 128
THETA = 10000.0
NEG = -60.0
F32 = mybir.dt.float32
BF16 = mybir.dt.bfloat16
FP8 = mybir.dt.float8e4
MULT = mybir.AluOpType.mult
ADD = mybir.AluOpType.add
SUB = mybir.AluOpType.subtract
EXP = mybir.ActivationFunctionType.Exp
LN = mybir.ActivationFunctionType.Ln
COPY = mybir.ActivationFunctionType.Copy
DR = mybir.MatmulPerfMode.DoubleRow

FP8_QK = True      # fp8 DoubleRow q/k projection (bf16 fallback if False)
SX, SW = 8.0, 64.0
_F = (SX * SW) if FP8_QK else 1.0
SCALE = float(1.0 / (_F * _F * np.sqrt(DH)))   # softmax exp input scale

# score tiles per head: ALLOWED[sq_half] = sk tiles; diagonal tiles are
# masked via the 0/1 tri tile, (t>=4, sh=1) also get the seq-len bias.
ALLOWED = {0: [0, 1, 2, 3], 1: [0, 1, 2, 3, 4, 5, 6, 7]}
PARTIAL = {(t, 0) for t in range(4)} | {(t, 1) for t in range(4, 8)}


def build_nc(reps=1):
    nc = bacc.Bacc()
    qk_dt = FP8 if FP8_QK else BF16
    x8_d = nc.dram_tensor("x8", [P, 16, S], qk_dt, kind="ExternalInput")
    xb_d = nc.dram_tensor("xb", [P, 16, S], BF16, kind="ExternalInput")
    w8_d = nc.dram_tensor("w8", [P, 16, 16, P], qk_dt, kind="ExternalInput")
    wv_d = nc.dram_tensor("wv", [P, 2, 16, 512], BF16, kind="ExternalInput")
    wo_d = nc.dram_tensor("wo", [P, 8, D], BF16, kind="ExternalInput")
    cs_d = nc.dram_tensor("cs", [P, 2, S], BF16, kind="ExternalInput")
    tri_d = nc.dram_tensor("tri", [P, P], BF16, kind="ExternalInput")
    bias_d = nc.dram_tensor("bias", [P, 4], F32, kind="ExternalInput")
    ones_d = nc.dram_tensor("ones", [P, P], BF16, kind="ExternalInput")
    outT_d = nc.dram_tensor("outT", [P, 16, S], BF16, kind="ExternalOutput")

    with tile.TileContext(nc) as tc:
      for rep in range(reps):
        with (
            tc.tile_pool(name="qkt", bufs=1) as qktp,      # [128,16,1024] bf16 32K/p
            tc.tile_pool(name="vsb", bufs=1) as vsbp,      # [128,8,1024] bf16 16K/p
            tc.tile_pool(name="cstb", bufs=1) as cstbp,    # tri/bias/ones consts
            tc.tile_pool(name="ps", bufs=4, space=bass.MemorySpace.PSUM) as psp,
        ):
            # Pin the ACT table to natural_log_exp_and_others (contains Exp,
            # Ln, Copy) so the auto-insertion pass doesn't ping-pong between
            # the exp-only and ln-only sets (~2.7us per reload).
            _tabs = list(hw_specs.get_activation_tables(nc.m.arch).keys())
            _ld = mybir.InstLoadActFuncSet(
                name=nc.get_next_instruction_name(), ins=[], outs=[],
                act_func_set_id=_tabs.index("natural_log_exp_and_others"),
            )
            _ld.engine = mybir.EngineType.Activation
            nc.scalar.add_instruction(_ld)

            qkT = qktp.tile([P, 16, S], BF16, tag="qkt")
            vsb = vsbp.tile([P, 8, DC], BF16, tag="vsb")
            tri_t = cstbp.tile([P, P], BF16, tag="tri")
            nc.scalar.dma_start(tri_t[:], tri_d[:])
            bias_t = cstbp.tile([P, 4], F32, tag="bias")
            nc.scalar.dma_start(bias_t[:], bias_d[:])
            ones_t = cstbp.tile([P, P], BF16, tag="ones")
            nc.scalar.dma_start(ones_t[:], ones_d[:])

            # ================= phase A: QKV projection + RoPE =================
            with (
                tc.tile_pool(name="x8", bufs=1) as x8p,      # fp8 16K/p (bf16 32K)
                tc.tile_pool(name="xb", bufs=1) as xbp,      # bf16 32K/p
                tc.tile_pool(name="wqk", bufs=1) as wqkp,    # [128,16,16,128] 32K/p fp8
                tc.tile_pool(name="wv", bufs=2) as wvp,      # [128,16,512] 16K/p bf16
                tc.tile_pool(name="cst", bufs=1) as cstp,    # cos/sin 4K/p
                tc.tile_pool(name="rope", bufs=2) as ropep,  # [128,1024] bf16 2K/p
                tc.tile_pool(name="psw", bufs=1, space=bass.MemorySpace.PSUM) as pswp,
                tc.tile_pool(name="psv", bufs=2, space=bass.MemorySpace.PSUM) as psvp,
            ):
                x8t = x8p.tile([P, 16, S], qk_dt, tag="x8")
                xbt = xbp.tile([P, 16, S], BF16, tag="xb")
                w8t = wqkp.tile([P, 16, 16, P], qk_dt, tag="wqk")
                cs_t = cstp.tile([P, 2, S], BF16, tag="cs")
                with tc.high_priority():
                    # cs first (small) so PE warmup matmuls can start, then
                    # the tensors gating the first q/k m-tiles.
                    nc.sync.dma_start(cs_t[:], cs_d[:])
                    nc.sync.dma_start(x8t[:], x8_d[:])
                    for quarter in range(4):
                        nc.scalar.dma_start(
                            w8t[:, 4 * quarter : 4 * (quarter + 1)],
                            w8_d[:, 4 * quarter : 4 * (quarter + 1)],
                        )
                    # PE warmup: ~10us of throwaway matmuls so the HAM clock
                    # gate reaches 8/8 and the DMA wait is masked; gated only
                    # on the small cs transfer. Output is never read.
                    warm_ps = pswp.tile([P, 512], F32, tag="psw")
                    for w in range(44):
                        nc.tensor.matmul(
                            warm_ps[:], cs_t[:, 0, 0:P], cs_t[:, 0, 0:512],
                            start=True, stop=True,
                        )
                for half in range(2):
                    nc.gpsimd.dma_start(
                        xbt[:, 8 * half : 8 * (half + 1), :],
                        xb_d[:, 8 * half : 8 * (half + 1), :],
                    )
                wvts = []
                for nhp in range(2):
                    wvt = wvp.tile([P, 16, 512], BF16, tag="wv", name=f"wv{nhp}")
                    nc.gpsimd.dma_start(wvt[:], wv_d[:, nhp])
                    wvts.append(wvt)

                # ---- q/k projection (m-tile mt: 0..7 = q heads, 8..15 = k heads)
                for mt in range(16):
                    wqk = w8t[:, mt]
                    ps0 = psp.tile([P, 512], F32, tag="ps", name=f"qk{mt}_0")
                    ps1 = psp.tile([P, 512], F32, tag="ps", name=f"qk{mt}_1")
                    if FP8_QK:
                        for kk in range(8):
                            lw = wqk[:, 2 * kk : 2 * kk + 2, :]
                            nc.tensor.matmul(
                                ps0[:], lw, x8t[:, 2 * kk : 2 * kk + 2, 0:512],
                                start=(kk == 0), stop=(kk == 7), perf_mode=DR,
                            )
                            nc.tensor.matmul(
                                ps1[:], lw, x8t[:, 2 * kk : 2 * kk + 2, 512:1024],
                                start=(kk == 0), stop=(kk == 7), perf_mode=DR,
                            )
                    else:
                        for kt in range(16):
                            nc.tensor.matmul(
                                ps0[:], wqk[:, kt, :], x8t[:, kt, 0:512],
                                start=(kt == 0), stop=(kt == 15),
                            )
                            nc.tensor.matmul(
                                ps1[:], wqk[:, kt, :], x8t[:, kt, 512:1024],
                                start=(kt == 0), stop=(kt == 15),
                            )
                    col = qkT[:, mt, :]
                    nc.scalar.activation(col[:, 0:512], ps0[:], COPY)
                    nc.scalar.activation(col[:, 512:1024], ps1[:], COPY)
                    # ---- RoPE in place on qkT[:, mt, :].
                    # rows 0..63 = even dh (xe), 64..127 = odd dh (xo):
                    #   new_e = xe*cos - xo*sin ; new_o = xo*cos + xe*sin
                    # (both TT inputs must share a base partition; only the
                    # output AP may be partition-shifted)
                    eng = nc.vector
                    tmp = ropep.tile([P, S], BF16, tag="rope")
                    eng.tensor_tensor(
                        tmp[0:64, :], col[64:128, :], cs_t[64:128, 1, :], op=MULT
                    )
                    eng.tensor_tensor(
                        tmp[64:128, :], col[0:64, :], cs_t[0:64, 1, :], op=MULT
                    )
                    eng.tensor_tensor(col[:], col[:], cs_t[:, 0, :], op=MULT)
                    eng.tensor_tensor(
                        col[0:64, :], col[0:64, :], tmp[0:64, :], op=SUB
                    )
                    eng.tensor_tensor(
                        col[64:128, :], col[64:128, :], tmp[64:128, :], op=ADD
                    )

                # ---- v projection: v[s, j] = sum_d xb[d, s] * Wv[j, d]
                for nhp in range(2):
                    wvt = wvts[nhp]
                    for st in range(8):
                        psv = psvp.tile([P, 512], F32, tag="psv")
                        for kt in range(16):
                            nc.tensor.matmul(
                                psv[:],
                                xbt[:, kt, P * st : P * (st + 1)],
                                wvt[:, kt, :],
                                start=(kt == 0),
                                stop=(kt == 15),
                            )
                        nc.scalar.copy(vsb[:, st, 512 * nhp : 512 * (nhp + 1)], psv[:])

            # ============ phase B: attention + output projection ============
            with (
                tc.tile_pool(name="ctx", bufs=1) as ctxp,    # [128,8,1024] bf16 16K/p
                tc.tile_pool(name="ex", bufs=6) as exps,     # [128,512] bf16 1K/p
                tc.tile_pool(name="rc", bufs=2) as rcp,      # [128,512] f32 2K/p
                tc.tile_pool(name="rcb", bufs=2) as rcbp,    # [128,512] bf16 1K/p
                tc.tile_pool(name="cu", bufs=3) as cup,      # [128,512] bf16 1K/p
                tc.tile_pool(name="wo", bufs=1) as wop,      # [128,8,2048] bf16 32K/p
                tc.tile_pool(name="ot", bufs=3) as otp,      # [128,512] bf16 1K/p
                tc.tile_pool(name="psc", bufs=2, space=bass.MemorySpace.PSUM) as pscp,
                tc.tile_pool(name="psd", bufs=2, space=bass.MemorySpace.PSUM) as psdp,
            ):
                ctxT = ctxp.tile([P, 8, S], BF16, tag="ctx")
                wo_t = wop.tile([P, 8, D], BF16, tag="wo")
                nc.sync.dma_start(wo_t[:], wo_d[:])

                # ---- attention, software-pipelined: scores issue LOOKAHEAD
                # items ahead of their exp/ctx/den so the PE never waits on
                # the ACT exp chain.
                work = []  # (h, sh, t, i, ntiles)
                for h in range(8):
                    for sh in range(2):
                        tiles = ALLOWED[sh]
                        for i, t in enumerate(tiles):
                            work.append((h, sh, t, i, len(tiles)))

                LOOKAHEAD = 3
                scs = {}
                groups = {}  # (h, sh) -> (ctx_ps, den_ps)

                def issue_score(j):
                    h, sh, t, i, _n = work[j]
                    partial = (t, sh) in PARTIAL
                    c0 = P * t - 512 * sh if partial else 0
                    sc = psp.tile([P, 512], F32, tag="ps")
                    nc.tensor.matmul(
                        sc[:, c0:512],
                        qkT[:, 8 + h, P * t : P * (t + 1)],
                        qkT[:, h, 512 * sh + c0 : 512 * (sh + 1)],
                        start=True,
                        stop=True,
                    )
                    scs[j] = sc

                for j in range(min(LOOKAHEAD, len(work))):
                    issue_score(j)
                for j, (h, sh, t, i, ntiles) in enumerate(work):
                    if j + LOOKAHEAD < len(work):
                        issue_score(j + LOOKAHEAD)
                    sc = scs.pop(j)
                    partial = (t, sh) in PARTIAL
                    c0 = P * t - 512 * sh if partial else 0
                    ex = exps.tile([P, 512], BF16, tag="ex")
                    bias = bias_t[:, t - 4 : t - 3] if (sh == 1 and t >= 4) else 0.0
                    nc.scalar.activation(
                        ex[:, c0:512], sc[:, c0:512], EXP, bias=bias, scale=SCALE
                    )
                    if partial:
                        nc.vector.tensor_tensor(
                            ex[:, c0 : c0 + P], ex[:, c0 : c0 + P], tri_t[:], op=MULT
                        )
                    if i == 0:
                        ctx_ps = pscp.tile([P, 512], F32, tag="psc", name=f"ctxps_{h}_{sh}")
                        den_ps = psdp.tile([P, 512], F32, tag="psd", name=f"denps_{h}_{sh}")
                        groups[(h, sh)] = (ctx_ps, den_ps)
                    ctx_ps, den_ps = groups[(h, sh)]
                    first, last = (i == 0), (i == ntiles - 1)
                    nc.tensor.matmul(
                        ctx_ps[:, c0:512],
                        vsb[:, t, DH * h : DH * (h + 1)],
                        ex[:, c0:512],
                        start=first,
                        stop=last,
                    )
                    nc.tensor.matmul(
                        den_ps[:, c0:512], ones_t[:], ex[:, c0:512], start=first, stop=last
                    )
                    if last:
                        # 1/den on ACT: rc = exp(-ln(den)); both fns share
                        # the natural_log_exp table set (no table switch).
                        # ctx_ps is evacuated bf16 by DVE, the normalize
                        # multiply runs on the otherwise-idle GpSimd (a
                        # PSUM-source fp32 TT on DVE measured ~2.5us).
                        lden = rcp.tile([P, 512], F32, tag="rc")
                        rc = rcbp.tile([P, 512], BF16, tag="rcb")
                        nc.scalar.activation(lden[:], den_ps[:], LN)
                        nc.scalar.activation(rc[:], lden[:], EXP, scale=-1.0)
                        cu = cup.tile([P, 512], BF16, tag="cu")
                        nc.vector.tensor_copy(cu[:], ctx_ps[:])
                        nc.gpsimd.tensor_tensor(
                            ctxT[:, h, 512 * sh : 512 * (sh + 1)],
                            cu[:],
                            rc[:],
                            op=MULT,
                        )

                # ---- output projection: outT[e, sq] = sum_d wo[d, e] * ctxT[d, sq]
                for me in range(16):
                    for sh in range(2):
                        po = psp.tile([P, 512], F32, tag="ps")
                        for kd in range(8):
                            nc.tensor.matmul(
                                po[:],
                                wo_t[:, kd, P * me : P * (me + 1)],
                                ctxT[:, kd, 512 * sh : 512 * (sh + 1)],
                                start=(kd == 0),
                                stop=(kd == 7),
                            )
                        ot = otp.tile([P, 512], BF16, tag="ot")
                        if (me + sh) % 2 == 0:
                            nc.scalar.copy(ot[:], po[:])
                        else:
                            nc.vector.tensor_copy(ot[:], po[:])
                        eng = nc.sync if sh == 0 else nc.scalar
                        eng.dma_start(
                            outT_d[:, me, 512 * sh : 512 * (sh + 1)], ot[:]
                        )
    nc.finalize()
    return nc


_NC_CACHE = None


def get_nc():
    global _NC_CACHE
    if _NC_CACHE is None:
        _NC_CACHE = build_nc()
    return _NC_CACHE


def _bf16(a):
    return np.ascontiguousarray(a.astype(ml_dtypes.bfloat16))


def _fp8(a):
    return np.ascontiguousarray(
        np.clip(a, -240.0, 240.0).astype(ml_dtypes.float8_e4m3)
    )


def make_in_maps(in_features, attention_mask, W_qkv, W_out):
    x = np.asarray(in_features, np.float32)
    am = np.asarray(attention_mask)
    Wqkv = np.asarray(W_qkv, np.float32)
    Wout = np.asarray(W_out, np.float32)
    seq_lens = am.astype(np.int64).sum(-1)

    perm = np.concatenate([np.arange(0, DH, 2), np.arange(1, DH, 2)])
    Wqh = Wqkv[0:D].reshape(H, DH, D)
    Wkh = Wqkv[D : 2 * D].reshape(H, DH, D)
    Wvh = Wqkv[2 * D : 3 * D].reshape(H, DH, D)

    half = DH // 2
    freq = THETA ** (-2.0 * np.arange(half, dtype=np.float64) / DH)
    ang = np.arange(S, dtype=np.float64)[:, None] * freq  # [S, 64]
    cosv = np.cos(ang).T.astype(np.float32)  # [64, S]
    sinv = np.sin(ang).T.astype(np.float32)
    cs = np.empty([P, 2, S], np.float32)
    cs[0:64, 0] = cosv
    cs[64:128, 0] = cosv
    cs[0:64, 1] = sinv
    cs[64:128, 1] = sinv
    cs = _bf16(cs)

    ones = np.ones([P, P], ml_dtypes.bfloat16)
    pp = np.arange(P)[:, None]
    cc = np.arange(P)[None, :]
    tri = _bf16((pp <= cc).astype(np.float32))  # 1 on/above diag (sk<=sq)

    cvt_qk = _fp8 if FP8_QK else _bf16
    sw = SW if FP8_QK else 1.0
    sx = SX if FP8_QK else 1.0

    in_maps = []
    for c in range(8):
        b, g = c // 2, c % 2
        hs = slice(g * HPC, (g + 1) * HPC)
        # q/k weights: [p, mt, kt, m]; mt 0..7 q heads, 8..15 k heads
        wq = Wqh[hs][:, perm, :].reshape(DC, D) * sw
        wk = Wkh[hs][:, perm, :].reshape(DC, D) * sw
        wcat = np.concatenate([wq, wk], 0)               # [2048, 2048] (m, d)
        w8 = cvt_qk(wcat.reshape(16, P, 16, P).transpose(3, 0, 2, 1))
        # v weights: [p, nhp, kt, j]
        wv = Wvh[hs].reshape(DC, D)                      # [1024, 2048] (j, d)
        wv8 = _bf16(wv.reshape(2, 512, 16, P).transpose(3, 0, 2, 1))
        # out weights: [p, kd, e]
        wo8 = _bf16(
            Wout[:, g * DC : (g + 1) * DC].reshape(D, 8, P).transpose(2, 1, 0)
        )
        # x: [p, kt, s]
        xT = x[b].T                                      # [D, S]
        x8 = cvt_qk((xT * sx).reshape(16, P, S).transpose(1, 0, 2))
        xb = _bf16(xT.reshape(16, P, S).transpose(1, 0, 2))

        sl = int(seq_lens[b])
        bias = np.zeros([P, 4], np.float32)
        for t in range(4, 8):
            bias[:, t - 4] = np.where(t * P + np.arange(P) >= sl, NEG, 0.0)
        in_maps.append(
            dict(x8=x8, xb=xb, w8=w8, wv=wv8, wo=wo8, cs=cs,
                 tri=tri, bias=bias, ones=ones)
        )
    return in_maps


def kernel(in_features, past_k, past_v, attention_mask, W_qkv, W_out):
    nc = get_nc()
    in_maps = make_in_maps(in_features, attention_mask, W_qkv, W_out)
    res = run_bass_kernel_spmd(nc, in_maps, core_ids=list(range(8)))
    out = np.empty((B, S, D), np.float32)
    for b in range(B):
        acc = None
        for g in range(2):
            o = res.results[2 * b + g]["outT"].astype(np.float32)
            o = o.transpose(1, 0, 2).reshape(D, S)       # [e, s]
            acc = o if acc is None else acc + o
        out[b] = acc.T
    return out


# revision 19
# speedup vs baseline: 1.5781x; 1.0917x over previous
"""Trainium2 Bass kernel for fused attention prefill (nn_Attn_50740743635107).

Reference computation (fp32):
  qkv = x @ W_qkv.T ; split q,k,v ; interleaved RoPE on q,k ;
  scores = q k^T / sqrt(dh) with causal+valid_k mask ; softmax ;
  ctx = attn @ v ; out = ctx @ W_out.T

Shapes: B=4, S=1024, D=2048, H=16, DH=128.

Sharding: 8 cores = 4 batches x 2 head-groups (8 heads each).
Each core computes a partial out^T for its (batch, head-group);
the host sums the two head-group partials per batch and transposes.

v2 design (vs fp32r v1):
- All operands bf16 except the q/k projection which runs fp8e4m3 with
  DoubleRow perf mode (2 k-tiles per matmul). Host bakes power-of-2
  scales (x*8, Wqk*64); the combined descale folds into the softmax
  exp's free scale operand.
- Host pre-arranges every DRAM tensor into its exact SBUF layout so all
  DMAs are contiguous.
- RoPE in bf16 on DVE/GpSimd (4 tensor_tensor ops per 128-row tile, 2x
  DVE mode); q/k rows pre-permuted so even/odd pairs become contiguous
  halves.
- Causal mask applied multiplicatively AFTER exp (0/1 bf16 tri tile) so
  exp reads score PSUM directly; ragged seq_len masking stays as a
  per-partition additive bias inside the exp activation.
- Softmax reciprocal via ACT ln + exp(-x) (both functions live in the
  natural_log_exp table set) instead of the slow DVE iterative divide.
- Denominators via an all-ones stationary matmul accumulated alongside
  ctx, giving a partition-broadcast denominator for the normalize.
"""

from contextlib import ExitStack

import numpy as np
import ml_dtypes

import concourse.bass as bass
from concourse import bacc
from concourse import hw_specs
import concourse.mybir as mybir
import concourse.tile as tile
from concourse.bass_utils import run_bass_kernel_spmd

B, S, D, H = 4, 1024, 2048, 16
DH = 128           # head dim
HPC = 8            # heads per core
DC = HPC * DH      # 1024: d-range per core
P = 128
THETA = 10000.0
NEG = -60.0
F32 = mybir.dt.float32
BF16 = mybir.dt.bfloat16
FP8 = mybir.dt.float8e4
MULT = mybir.AluOpType.mult
ADD = mybir.AluOpType.add
SUB = mybir.AluOpType.subtract
EXP = mybir.ActivationFunctionType.Exp
LN = mybir.ActivationFunctionType.Ln
COPY = mybir.ActivationFunctionType.Copy
DR = mybir.MatmulPerfMode.DoubleRow

FP8_QK = True      # fp8 DoubleRow q/k projection (bf16 fallback if False)
SX, SW = 8.0, 64.0
_F = (SX * SW) if FP8_QK else 1.0
SCALE = float(1.0 / (_F * _F * np.sqrt(DH)))   # softmax exp input scale

# score tiles per head: ALLOWED[sq_half] = sk tiles; diagonal tiles are
# masked via the 0/1 tri tile, (t>=4, sh=1) also get the seq-len bias.
ALLOWED = {0: [0, 1, 2, 3], 1: [0, 1, 2, 3, 4, 5, 6, 7]}
PARTIAL = {(t, 0) for t in range(4)} | {(t, 1) for t in range(4, 8)}


def build_nc(reps=1):
    nc = bacc.Bacc()
    qk_dt = FP8 if FP8_QK else BF16
    x8_d = nc.dram_tensor("x8", [P, 16, S], qk_dt, kind="ExternalInput")
    xb_d = nc.dram_tensor("xb", [P, 16, S], BF16, kind="ExternalInput")
    w8_d = nc.dram_tensor("w8", [P, 16, 16, P], qk_dt, kind="ExternalInput")
    wv_d = nc.dram_tensor("wv", [P, 2, 16, 512], BF16, kind="ExternalInput")
    wo_d = nc.dram_tensor("wo", [P, 8, D], BF16, kind="ExternalInput")
    cs_d = nc.dram_tensor("cs", [P, 2, S], BF16, kind="ExternalInput")
    tri_d = nc.dram_tensor("tri", [P, P], BF16, kind="ExternalInput")
    bias_d = nc.dram_tensor("bias", [P, 4], F32, kind="ExternalInput")
    ones_d = nc.dram_tensor("ones", [P, P], BF16, kind="ExternalInput")
    outT_d = nc.dram_tensor("outT", [P, 16, S], BF16, kind="ExternalOutput")

    with tile.TileContext(nc) as tc:
      for rep in range(reps):
        with ExitStack() as es:
            pool = lambda **kw: es.enter_context(tc.tile_pool(**kw))
            qktp = pool(name="qkt", bufs=1)    # [128,16,1024] bf16 32K/p
            vsbp = pool(name="vsb", bufs=1)    # [128,8,1024] bf16 16K/p
            xbp = pool(name="xb", bufs=1)      # bf16 32K/p
            wvp = pool(name="wv", bufs=2)      # [128,16,512] 16K/p bf16
            cstbp = pool(name="cstb", bufs=1)  # tri/bias/ones consts
            gtp = pool(name="gt", bufs=1)      # dma-gate scratch
            psp = pool(name="ps", bufs=4, space=bass.MemorySpace.PSUM)
            qkT = qktp.tile([P, 16, S], BF16, tag="qkt")
            vsb = vsbp.tile([P, 8, DC], BF16, tag="vsb")
            xbt = xbp.tile([P, 16, S], BF16, tag="xb")
            tri_t = cstbp.tile([P, P], BF16, tag="tri")
            nc.gpsimd.dma_start(tri_t[:], tri_d[:])
            bias_t = cstbp.tile([P, 4], F32, tag="bias")
            nc.gpsimd.dma_start(bias_t[:], bias_d[:])
            ones_t = cstbp.tile([P, P], BF16, tag="ones")
            nc.gpsimd.dma_start(ones_t[:], ones_d[:])

            # ================= phase A1: q/k projection + RoPE ===============
            with ExitStack() as esA:
                poolA = lambda **kw: esA.enter_context(tc.tile_pool(**kw))
                x8p = poolA(name="x8", bufs=1)     # fp8 16K/p (bf16 32K)
                wqkp = poolA(name="wqk", bufs=1)   # [128,16,16,128] 32K/p
                cstp = poolA(name="cst", bufs=1)   # cos/sin 4K/p
                ropep = poolA(name="rope", bufs=2) # [128,1024] bf16 2K/p
                pswp = poolA(name="psw", bufs=1, space=bass.MemorySpace.PSUM)
                x8t = x8p.tile([P, 16, S], qk_dt, tag="x8")
                w8t = wqkp.tile([P, 16, 16, P], qk_dt, tag="wqk")
                cs_t = cstp.tile([P, 2, S], BF16, tag="cs")
                with tc.high_priority():
                    # cs first (small) so PE warmup matmuls can start, then
                    # the tensors gating the first q/k m-tiles. x on sync,
                    # weights on the scalar HWDGE queue, in parallel.
                    nc.sync.dma_start(cs_t[:], cs_d[:])
                    nc.sync.dma_start(x8t[:], x8_d[:])
                    for quarter in range(4):
                        nc.scalar.dma_start(
                            w8t[:, 4 * quarter : 4 * (quarter + 1)],
                            w8_d[:, 4 * quarter : 4 * (quarter + 1)],
                        )
                    # PE warmup: ~14us of throwaway matmuls so the HAM clock
                    # gate reaches 8/8 and the x/w DMA wait is masked; gated
                    # only on the small cs transfer. Output is never read.
                    warm_ps = pswp.tile([P, 512], F32, tag="psw")
                    for w in range(64):
                        nc.tensor.matmul(
                            warm_ps[:], cs_t[:, 0, 0:P], cs_t[:, 0, 0:512],
                            start=True, stop=True,
                        )

                # ---- q/k projection (m-tile mt: 0..7 = q heads, 8..15 = k)
                for mt in range(16):
                    wqk = w8t[:, mt]
                    ps0 = psp.tile([P, 512], F32, tag="ps", name=f"qk{mt}_0")
                    ps1 = psp.tile([P, 512], F32, tag="ps", name=f"qk{mt}_1")
                    if FP8_QK:
                        for kk in range(8):
                            lw = wqk[:, 2 * kk : 2 * kk + 2, :]
                            nc.tensor.matmul(
                                ps0[:], lw, x8t[:, 2 * kk : 2 * kk + 2, 0:512],
                                start=(kk == 0), stop=(kk == 7), perf_mode=DR,
                            )
                            nc.tensor.matmul(
                                ps1[:], lw, x8t[:, 2 * kk : 2 * kk + 2, 512:1024],
                                start=(kk == 0), stop=(kk == 7), perf_mode=DR,
                            )
                    else:
                        for kt in range(16):
                            nc.tensor.matmul(
                                ps0[:], wqk[:, kt, :], x8t[:, kt, 0:512],
                                start=(kt == 0), stop=(kt == 15),
                            )
                            nc.tensor.matmul(
                                ps1[:], wqk[:, kt, :], x8t[:, kt, 512:1024],
                                start=(kt == 0), stop=(kt == 15),
                            )
                    col = qkT[:, mt, :]
                    nc.scalar.activation(col[:, 0:512], ps0[:], COPY)
                    nc.scalar.activation(col[:, 512:1024], ps1[:], COPY)
                    # ---- RoPE in place on qkT[:, mt, :].
                    # rows 0..63 = even dh (xe), 64..127 = odd dh (xo):
                    #   new_e = xe*cos - xo*sin ; new_o = xo*cos + xe*sin
                    # (both TT inputs must share a base partition; only the
                    # output AP may be partition-shifted)
                    tmp = ropep.tile([P, S], BF16, tag="rope")
                    nc.vector.tensor_tensor(
                        tmp[0:64, :], col[64:128, :], cs_t[64:128, 1, :], op=MULT
                    )
                    nc.vector.tensor_tensor(
                        tmp[64:128, :], col[0:64, :], cs_t[0:64, 1, :], op=MULT
                    )
                    nc.vector.tensor_tensor(col[:], col[:], cs_t[:, 0, :], op=MULT)
                    nc.vector.tensor_tensor(
                        col[0:64, :], col[0:64, :], tmp[0:64, :], op=SUB
                    )
                    nc.vector.tensor_tensor(
                        col[64:128, :], col[64:128, :], tmp[64:128, :], op=ADD
                    )

            # Pools for attention + out-projection, created only after the
            # A1 pools have released their SBUF (the allocator reserves pool
            # space in creation order for the pool's scope lifetime).
            ctxp = pool(name="ctx", bufs=1)    # [128,8,1024] bf16 16K/p
            exps = pool(name="ex", bufs=6)     # [128,512] bf16 1K/p
            rcp = pool(name="rc", bufs=3)      # [128,512] f32 2K/p
            cup = pool(name="cu", bufs=3)      # [128,512] bf16 1K/p
            wop = pool(name="wo", bufs=1)      # [128,8,2048] bf16 32K/p
            otp = pool(name="ot", bufs=3)      # [128,512] bf16 1K/p
            pscp = pool(name="psc", bufs=2, space=bass.MemorySpace.PSUM)
            psdp = pool(name="psd", bufs=2, space=bass.MemorySpace.PSUM)

            # Late bulk DMAs (xb/wv/wo), gated on mt0 being roped so they do
            # not steal HBM bandwidth from the startup-critical x8/w8 loads.
            gate = gtp.tile([1, 2], BF16, tag="gt")
            nc.gpsimd.tensor_copy(gate[:], qkT[0:1, 0, 0:2])
            for half in range(2):
                nc.gpsimd.dma_start(
                    xbt[:, 8 * half : 8 * (half + 1), :],
                    xb_d[:, 8 * half : 8 * (half + 1), :],
                )
            wvts = []
            for nhp in range(2):
                wvt = wvp.tile([P, 16, 512], BF16, tag="wv", name=f"wv{nhp}")
                nc.gpsimd.dma_start(wvt[:], wv_d[:, nhp])
                wvts.append(wvt)
            ctxT = ctxp.tile([P, 8, S], BF16, tag="ctx")
            wo_t = wop.tile([P, 8, D], BF16, tag="wo")
            nc.gpsimd.dma_start(wo_t[:], wo_d[:])

            # ---- v projection block: v[s, j] = sum_d xb[d, s] * Wv[j, d]
            def emit_v_block(nhp, st):
                psv = psp.tile([P, 512], F32, tag="ps", name=f"v{nhp}_{st}")
                for kt in range(16):
                    nc.tensor.matmul(
                        psv[:],
                        xbt[:, kt, P * st : P * (st + 1)],
                        wvts[nhp][:, kt, :],
                        start=(kt == 0),
                        stop=(kt == 15),
                    )
                nc.scalar.copy(vsb[:, st, 512 * nhp : 512 * (nhp + 1)], psv[:])

            # ---- attention machinery, software-pipelined: scores issue
            # `la` items ahead of their exp/ctx/den so the PE never waits on
            # the ACT exp chain.
            class Attn:
                def __init__(self, heads, la):
                    self.work = []
                    for h in heads:
                        for sh in range(2):
                            tiles = ALLOWED[sh]
                            for i, t in enumerate(tiles):
                                self.work.append((h, sh, t, i, len(tiles)))
                    self.la = la
                    self.scs = {}
                    self.groups = {}
                    self.nissued = 0
                    self.ndone = 0

                def issue_score(self):
                    j = self.nissued
                    h, sh, t, i, _n = self.work[j]
                    partial = (t, sh) in PARTIAL
                    c0 = P * t - 512 * sh if partial else 0
                    sc = psp.tile([P, 512], F32, tag="ps")
                    nc.tensor.matmul(
                        sc[:, c0:512],
                        qkT[:, 8 + h, P * t : P * (t + 1)],
                        qkT[:, h, 512 * sh + c0 : 512 * (sh + 1)],
                        start=True,
                        stop=True,
                    )
                    self.scs[j] = sc
                    self.nissued += 1

                def advance(self, n):
                    for _ in range(n):
                        j = self.ndone
                        if j >= len(self.work):
                            return
                        while self.nissued < min(j + 1 + self.la, len(self.work)):
                            self.issue_score()
                        h, sh, t, i, ntiles = self.work[j]
                        sc = self.scs.pop(j)
                        partial = (t, sh) in PARTIAL
                        c0 = P * t - 512 * sh if partial else 0
                        ex = exps.tile([P, 512], BF16, tag="ex")
                        bias = (
                            bias_t[:, t - 4 : t - 3]
                            if (sh == 1 and t >= 4)
                            else 0.0
                        )
                        nc.scalar.activation(
                            ex[:, c0:512], sc[:, c0:512], EXP,
                            bias=bias, scale=SCALE,
                        )
                        if partial:
                            nc.vector.tensor_tensor(
                                ex[:, c0 : c0 + P], ex[:, c0 : c0 + P],
                                tri_t[:], op=MULT,
                            )
                        if i == 0:
                            ctx_ps = pscp.tile(
                                [P, 512], F32, tag="psc", name=f"ctxps_{h}_{sh}"
                            )
                            den_ps = psdp.tile(
                                [P, 512], F32, tag="psd", name=f"denps_{h}_{sh}"
                            )
                            self.groups[(h, sh)] = (ctx_ps, den_ps)
                        ctx_ps, den_ps = self.groups[(h, sh)]
                        first, last = (i == 0), (i == ntiles - 1)
                        nc.tensor.matmul(
                            ctx_ps[:, c0:512],
                            vsb[:, t, DH * h : DH * (h + 1)],
                            ex[:, c0:512],
                            start=first,
                            stop=last,
                        )
                        nc.tensor.matmul(
                            den_ps[:, c0:512], ones_t[:], ex[:, c0:512],
                            start=first, stop=last,
                        )
                        if last:
                            # 1/den via the fast custom-DVE reciprocal
                            # (~51 ULP, plenty for a softmax denominator);
                            # ctx_ps evacuates bf16 on DVE and the normalize
                            # multiply also runs on DVE (a direct PSUM-source
                            # fp32 TT measured ~2.5us; this chain is ~2.1us).
                            rc = rcp.tile([P, 512], F32, tag="rc")
                            nc.vector.reciprocal_approx_fast(rc[:], den_ps[:])
                            cu = cup.tile([P, 512], BF16, tag="cu")
                            nc.vector.tensor_copy(cu[:], ctx_ps[:])
                            nc.vector.tensor_tensor(
                                ctxT[:, h, 512 * sh : 512 * (sh + 1)],
                                cu[:],
                                rc[:],
                                op=MULT,
                            )
                        self.ndone += 1

            # v nhp0 first: heads 0..3 only need vsb columns 0..511.
            for st in range(8):
                emit_v_block(0, st)
            # interleave attention (h 0..3) with the nhp1 v blocks: the
            # attention inner loop is LDWEIGHTS-bound (3 stationaries per
            # item) while the v blocks are matmul-streaming-bound, so mixing
            # them keeps the PE dense.
            at1 = Attn(heads=range(4), la=2)
            for st in range(8):
                emit_v_block(1, st)
                at1.advance(12)
            at1.advance(len(at1.work))
            at2 = Attn(heads=range(4, 8), la=3)
            at2.advance(len(at2.work))

            # ---- output projection: outT[e, s] = sum_d wo[d, e] * ctxT[d, s]
            dmaq = [nc.sync, nc.scalar, nc.gpsimd]
            for me in range(16):
                for sh in range(2):
                    po = psp.tile([P, 512], F32, tag="ps")
                    for kd in range(8):
                        nc.tensor.matmul(
                            po[:],
                            wo_t[:, kd, P * me : P * (me + 1)],
                            ctxT[:, kd, 512 * sh : 512 * (sh + 1)],
                            start=(kd == 0),
                            stop=(kd == 7),
                        )
                    ot = otp.tile([P, 512], BF16, tag="ot")
                    if (me + sh) % 2 == 0:
                        nc.scalar.copy(ot[:], po[:])
                    else:
                        nc.vector.tensor_copy(ot[:], po[:])
                    eng = dmaq[(2 * me + sh) % 3]
                    eng.dma_start(
                        outT_d[:, me, 512 * sh : 512 * (sh + 1)], ot[:]
                    )
    nc.finalize()
    return nc


_NC_CACHE = None


def get_nc():
    global _NC_CACHE
    if _NC_CACHE is None:
        _NC_CACHE = build_nc()
    return _NC_CACHE


def _bf16(a):
    return np.ascontiguousarray(a.astype(ml_dtypes.bfloat16))


def _fp8(a):
    return np.ascontiguousarray(
        np.clip(a, -240.0, 240.0).astype(ml_dtypes.float8_e4m3)
    )


def make_in_maps(in_features, attention_mask, W_qkv, W_out):
    x = np.asarray(in_features, np.float32)
    am = np.asarray(attention_mask)
    Wqkv = np.asarray(W_qkv, np.float32)
    Wout = np.asarray(W_out, np.float32)
    seq_lens = am.astype(np.int64).sum(-1)

    perm = np.concatenate([np.arange(0, DH, 2), np.arange(1, DH, 2)])
    Wqh = Wqkv[0:D].reshape(H, DH, D)
    Wkh = Wqkv[D : 2 * D].reshape(H, DH, D)
    Wvh = Wqkv[2 * D : 3 * D].reshape(H, DH, D)

    half = DH // 2
    freq = THETA ** (-2.0 * np.arange(half, dtype=np.float64) / DH)
    ang = np.arange(S, dtype=np.float64)[:, None] * freq  # [S, 64]
    cosv = np.cos(ang).T.astype(np.float32)  # [64, S]
    sinv = np.sin(ang).T.astype(np.float32)
    cs = np.empty([P, 2, S], np.float32)
    cs[0:64, 0] = cosv
    cs[64:128, 0] = cosv
    cs[0:64, 1] = sinv
    cs[64:128, 1] = sinv
    cs = _bf16(cs)

    ones = np.ones([P, P], ml_dtypes.bfloat16)
    pp = np.arange(P)[:, None]
    cc = np.arange(P)[None, :]
    tri = _bf16((pp <= cc).astype(np.float32))  # 1 on/above diag (sk<=sq)

    cvt_qk = _fp8 if FP8_QK else _bf16
    sw = SW if FP8_QK else 1.0
    sx = SX if FP8_QK else 1.0

    in_maps = []
    for c in range(8):
        b, g = c // 2, c % 2
        hs = slice(g * HPC, (g + 1) * HPC)
        # q/k weights: [p, mt, kt, m]; mt 0..7 q heads, 8..15 k heads
        wq = Wqh[hs][:, perm, :].reshape(DC, D) * sw
        wk = Wkh[hs][:, perm, :].reshape(DC, D) * sw
        wcat = np.concatenate([wq, wk], 0)               # [2048, 2048] (m, d)
        w8 = cvt_qk(wcat.reshape(16, P, 16, P).transpose(3, 0, 2, 1))
        # v weights: [p, nhp, kt, j]
        wv = Wvh[hs].reshape(DC, D)                      # [1024, 2048] (j, d)
        wv8 = _bf16(wv.reshape(2, 512, 16, P).transpose(3, 0, 2, 1))
        # out weights: [p, kd, e]
        wo8 = _bf16(
            Wout[:, g * DC : (g + 1) * DC].reshape(D, 8, P).transpose(2, 1, 0)
        )
        # x: [p, kt, s]
        xT = x[b].T                                      # [D, S]
        x8 = cvt_qk((xT * sx).reshape(16, P, S).transpose(1, 0, 2))
        xb = _bf16(xT.reshape(16, P, S).transpose(1, 0, 2))

        sl = int(seq_lens[b])
        bias = np.zeros([P, 4], np.float32)
        for t in range(4, 8):
            bias[:, t - 4] = np.where(t * P + np.arange(P) >= sl, NEG, 0.0)
        in_maps.append(
            dict(x8=x8, xb=xb, w8=w8, wv=wv8, wo=wo8, cs=cs,
                 tri=tri, bias=bias, ones=ones)
        )
    return in_maps


def kernel(in_features, past_k, past_v, attention_mask, W_qkv, W_out):
    nc = get_nc()
    in_maps = make_in_maps(in_features, attention_mask, W_qkv, W_out)
    res = run_bass_kernel_spmd(nc, in_maps, core_ids=list(range(8)))
    out = np.empty((B, S, D), np.float32)
    for b in range(B):
        acc = None
        for g in range(2):
            o = res.results[2 * b + g]["outT"].astype(np.float32)
            o = o.transpose(1, 0, 2).reshape(D, S)       # [e, s]
            acc = o if acc is None else acc + o
        out[b] = acc.T
    return out
